# revision 3
# baseline (speedup 1.0000x reference)
"""Multi-head attention (RoPE + SDPA + output projection) on 8 Trainium2 cores.

Problem: nn_Attention_80152679678101
  x[2,2048,2048] @ w_qkv.T -> rope(q,k) -> softmax(q k^T/sqrt(128)) v -> @ w_proj.T + b

Sharding: core c -> (batch b = c//4, head-group g = c%4, 4 heads each).
Tensor-parallel heads within each 4-core batch group; the output projection is
computed over local head-dims only and summed with a ReduceScatter(add), which
also scatters the output rows o across the group (rank r returns y^T rows
[512r, 512r+512) of its batch).

Dataflow is fully transposed so every matmul has its contraction dim on SBUF
partitions with no on-chip transposes: the host feeds x^T, w_qkv_slice^T and
w_proj_slice^T (bf16). Stages per core:
  A) qkv^T: Q^T,K^T as [head_dim, n] (lhsT=w^T, rhs=x^T); V as [n, head_dim]
     (lhsT=x^T, rhs=w_v^T)
  B) RoPE on Q^T/K^T: half-swap via SBUF->SBUF DMA + 3 DVE ops against
     host-precomputed cos/sin tables (sign folded into the sin table)
  C) per head: S^T = K^T-tiles.T @ Q^T (PE) -> exp via ACT (1/sqrt(128) scale
     folded; no max-subtraction, scores are ~N(0,1) so fp32 exp is safe) ->
     softmax denominators via an all-ones stationary matmul (yields l[q]
     replicated across all 128 partitions) -> O'^T = V.T @ P^T -> DVE
     reciprocal + scale
  D) y_partial^T[o,q] = w_proj_slice^T.T @ A_local^T + b/4 -> ReduceScatter
"""

import os

# Never attempt NTFF tracing unless a dev harness explicitly opts in: the
# trace path uploads artifacts to S3, which is unavailable when grading.
if "KERNEL_ALLOW_TRACE" not in os.environ:
    os.environ["BASS_NEVER_TRACE"] = "1"

from contextlib import ExitStack
from dataclasses import dataclass

import ml_dtypes
import numpy as np

import concourse.bass as bass
import concourse.mybir as mybir
import concourse.tile as tile
from concourse import bacc
from concourse.bass_utils import run_bass_kernel_spmd

BF16 = mybir.dt.bfloat16
FP32 = mybir.dt.float32
AF = mybir.ActivationFunctionType

NCORES = 8
GS = 4  # tensor-parallel group size (cores per batch)
REPLICA_GROUPS = [[0, 1, 2, 3], [4, 5, 6, 7]]
P = 128  # SBUF partitions
ROPE_BASE = 10000.0


@dataclass(frozen=True)
class Cfg:
    B: int = 2
    N: int = 2048  # sequence length
    D: int = 2048  # model dim
    H: int = 16  # total heads

    @property
    def HD(self):  # head dim
        return self.D // self.H

    @property
    def G(self):  # heads per core
        return self.H // GS

    @property
    def E(self):  # local qkv output rows
        return 3 * self.G * self.HD

    @property
    def KT(self):  # contraction tiles over D
        return self.D // P

    @property
    def SEQT(self):  # sequence tiles of 128
        return self.N // P

    @property
    def NT(self):  # matmul moving free-dim tile
        return min(512, self.N)

    @property
    def QT(self):  # moving-dim tiles over N
        return self.N // self.NT

    @property
    def OT(self):  # output-projection row tiles
        return self.D // P


FULL = Cfg()


def build(cfg: Cfg) -> bass.Bass:
    assert cfg.HD == P, "rope/half-swap layout assumes head_dim == 128"
    G, E, KT, SEQT, NT, QT, OT = (
        cfg.G, cfg.E, cfg.KT, cfg.SEQT, cfg.NT, cfg.QT, cfg.OT,
    )
    N, D = cfg.N, cfg.D
    VOFF = 2 * G * P  # column offset of the v block in wqkvT
    scale = 1.0 / float(np.sqrt(cfg.HD))

    nc = bacc.Bacc(
        "TRN2", target_bir_lowering=False, debug=False, num_devices=NCORES
    )

    xT = nc.dram_tensor("xT", [D, N], BF16, kind="ExternalInput")
    wqkvT = nc.dram_tensor("wqkvT", [D, E], BF16, kind="ExternalInput")
    wprojT = nc.dram_tensor("wprojT", [G * P, D], BF16, kind="ExternalInput")
    bias4 = nc.dram_tensor("bias4", [D], FP32, kind="ExternalInput")
    cosT = nc.dram_tensor("cosT", [P, N], BF16, kind="ExternalInput")
    sinT = nc.dram_tensor("sinT", [P, N], BF16, kind="ExternalInput")
    out = nc.dram_tensor("out", [D // GS, N], FP32, kind="ExternalOutput")

    with tile.TileContext(nc) as tc, ExitStack() as ctx:
        dram = ctx.enter_context(tc.tile_pool(name="dram", bufs=1, space="DRAM"))
        const = ctx.enter_context(tc.tile_pool(name="const", bufs=1))

        cos_sb = const.tile([P, N], BF16)
        sin_sb = const.tile([P, N], BF16)
        ones_sb = const.tile([P, P], BF16)
        bias_sb = const.tile([P, OT], FP32)
        nc.sync.dma_start(cos_sb[:], cosT[:])
        nc.sync.dma_start(sin_sb[:], sinT[:])
        nc.vector.memset(ones_sb[:], 1.0)
        nc.sync.dma_start(bias_sb[:], bias4.ap().rearrange("(t p) -> p t", p=P))

        # live through stages A-C
        qk_pool = ctx.enter_context(tc.tile_pool(name="qk", bufs=1))
        v_pool = ctx.enter_context(tc.tile_pool(name="v", bufs=1))
        qt_sb = [qk_pool.tile([P, N], BF16, name=f"q_h{j}") for j in range(G)]
        kt_sb = [qk_pool.tile([P, N], BF16, name=f"k_h{j}") for j in range(G)]
        v_sb = v_pool.tile([P, SEQT, G * P], BF16)

        # ---- stage A: qkv projection (+ rope fused into the epilogue) ----
        with (
            tc.tile_pool(name="inw", bufs=1) as in_pool,
            tc.tile_pool(name="rope", bufs=3) as rope_pool,
            tc.tile_pool(name="ps_a", bufs=4, space="PSUM") as ps_a,
        ):
            xT_sb = [in_pool.tile([P, N], BF16, name=f"xT{k}") for k in range(KT)]
            wq_sb = [in_pool.tile([P, E], BF16, name=f"wq{k}") for k in range(KT)]
            for k in range(KT):
                nc.sync.dma_start(xT_sb[k][:], xT[k * P : (k + 1) * P, :])
                nc.sync.dma_start(wq_sb[k][:], wqkvT[k * P : (k + 1) * P, :])

            # A2: V natural layout [n, G*HD]
            for s in range(SEQT):
                ps = ps_a.tile([P, G * P], FP32, name="ps_v", tag="ps")
                for k in range(KT):
                    nc.tensor.matmul(
                        ps[:],
                        xT_sb[k][:, s * P : (s + 1) * P],
                        wq_sb[k][:, VOFF : VOFF + G * P],
                        start=(k == 0),
                        stop=(k == KT - 1),
                    )
                nc.scalar.activation(v_sb[:, s, :], ps[:], AF.Copy)

            # A1: Q^T / K^T per head-dim tile, rope epilogue per NT chunk
            for e in range(2 * G):
                dst = qt_sb[e] if e < G else kt_sb[e - G]
                for q in range(QT):
                    ps = ps_a.tile([P, NT], FP32, name="ps_qk", tag="ps")
                    for k in range(KT):
                        nc.tensor.matmul(
                            ps[:],
                            wq_sb[k][:, e * P : (e + 1) * P],
                            xT_sb[k][:, q * NT : (q + 1) * NT],
                            start=(k == 0),
                            stop=(k == KT - 1),
                        )
                    sl = slice(q * NT, (q + 1) * NT)
                    raw = rope_pool.tile([P, NT], FP32, name="raw")
                    nc.scalar.activation(raw[:], ps[:], AF.Copy)
                    # rotate-half: swp = [raw[64:], raw[:64]]
                    swp = rope_pool.tile([P, NT], FP32, name="swp")
                    h = P // 2
                    nc.sync.dma_start(swp[0:h, :], raw[h:P, :])
                    nc.sync.dma_start(swp[h:P, :], raw[0:h, :])
                    tmp = rope_pool.tile([P, NT], FP32, name="tmp")
                    nc.vector.tensor_mul(tmp[:], swp[:], sin_sb[:, sl])
                    nc.vector.tensor_mul(raw[:], raw[:], cos_sb[:, sl])
                    nc.vector.tensor_add(dst[:, sl], raw[:], tmp[:])

        # ---- stage C: attention per head ----
        at_pool = ctx.enter_context(tc.tile_pool(name="at", bufs=1))
        at_sb = [at_pool.tile([P, N], BF16, name=f"at_h{j}") for j in range(G)]
        with (
            tc.tile_pool(name="pt", bufs=1) as pt_pool,
            tc.tile_pool(name="rb", bufs=2) as rb_pool,
            tc.tile_pool(name="ps_s", bufs=4, space="PSUM") as ps_s,
            tc.tile_pool(name="ps_l", bufs=2, space="PSUM") as ps_l,
            tc.tile_pool(name="ps_o", bufs=2, space="PSUM") as ps_o,
        ):
            for j in range(G):
                pt = pt_pool.tile([P, SEQT, N], BF16, name="pt", tag="pt")
                # scores S^T[k, q] blocks + exp
                for s in range(SEQT):
                    for q in range(QT):
                        ps = ps_s.tile([P, NT], FP32, name="ps_sc", tag="sc")
                        nc.tensor.matmul(
                            ps[:],
                            kt_sb[j][:, s * P : (s + 1) * P],
                            qt_sb[j][:, q * NT : (q + 1) * NT],
                            start=True,
                            stop=True,
                        )
                        nc.scalar.activation(
                            pt[:, s, q * NT : (q + 1) * NT], ps[:], AF.Exp,
                            scale=scale,
                        )
                # denominators (ones-matmul; result replicated over partitions)
                # and O'^T accumulation
                for q in range(QT):
                    psl = ps_l.tile([P, NT], FP32, name="ps_lb", tag="lb")
                    pso = ps_o.tile([P, NT], FP32, name="ps_ov", tag="ov")
                    for s in range(SEQT):
                        nc.tensor.matmul(
                            psl[:],
                            ones_sb[:],
                            pt[:, s, q * NT : (q + 1) * NT],
                            start=(s == 0),
                            stop=(s == SEQT - 1),
                        )
                        nc.tensor.matmul(
                            pso[:],
                            v_sb[:, s, j * P : (j + 1) * P],
                            pt[:, s, q * NT : (q + 1) * NT],
                            start=(s == 0),
                            stop=(s == SEQT - 1),
                        )
                    rb = rb_pool.tile([P, NT], FP32, name="rb")
                    nc.vector.reciprocal(rb[:], psl[:])
                    nc.vector.tensor_mul(
                        at_sb[j][:, q * NT : (q + 1) * NT], pso[:], rb[:]
                    )

        # ---- stage D: output projection + ReduceScatter ----
        y_part = dram.tile([D, N], FP32)
        y_sc = dram.tile([D // GS, N], FP32)
        with (
            tc.tile_pool(name="wp", bufs=1) as wp_pool,
            tc.tile_pool(name="ystg", bufs=4) as y_pool,
            tc.tile_pool(name="ps_y", bufs=4, space="PSUM") as ps_y,
        ):
            wp_sb = [wp_pool.tile([P, D], BF16, name=f"wp{j}") for j in range(G)]
            for j in range(G):
                nc.sync.dma_start(wp_sb[j][:], wprojT[j * P : (j + 1) * P, :])
            for o in range(OT):
                for q in range(QT):
                    ps = ps_y.tile([P, NT], FP32, name="ps_yt", tag="y")
                    for j in range(G):
                        nc.tensor.matmul(
                            ps[:],
                            wp_sb[j][:, o * P : (o + 1) * P],
                            at_sb[j][:, q * NT : (q + 1) * NT],
                            start=(j == 0),
                            stop=(j == G - 1),
                        )
                    ystg = y_pool.tile([P, NT], FP32, name="ystg")
                    nc.scalar.activation(
                        ystg[:], ps[:], AF.Identity, bias=bias_sb[:, o : o + 1]
                    )
                    nc.sync.dma_start(
                        y_part[o * P : (o + 1) * P, q * NT : (q + 1) * NT],
                        ystg[:],
                    )

        nc.gpsimd.collective_compute(
            "ReduceScatter",
            mybir.AluOpType.add,
            replica_groups=REPLICA_GROUPS,
            ins=[y_part[:]],
            outs=[y_sc[:]],
        )
        with tc.tile_pool(name="obnc", bufs=2) as o_pool:
            for t in range(D // GS // P):
                ob = o_pool.tile([P, N], FP32, name="ob")
                nc.sync.dma_start(ob[:], y_sc[t * P : (t + 1) * P, :])
                nc.sync.dma_start(out[t * P : (t + 1) * P, :], ob[:])

    nc.compile()
    return nc


def _rope_tables(cfg: Cfg):
    hd = cfg.HD
    inv_freq = 1.0 / (
        ROPE_BASE ** (np.arange(0, hd, 2, dtype=np.float32) / np.float32(hd))
    )
    ang = np.arange(cfg.N, dtype=np.float32)[:, None] * inv_freq[None, :]  # [N, hd/2]
    c = np.cos(ang).T  # [hd/2, N]
    s = np.sin(ang).T
    cosT = np.concatenate([c, c], axis=0)
    sinT = np.concatenate([-s, s], axis=0)
    return (
        np.ascontiguousarray(cosT).astype(ml_dtypes.bfloat16),
        np.ascontiguousarray(sinT).astype(ml_dtypes.bfloat16),
    )


def prepare_in_maps(x, w_qkv, w_proj, b_proj, cfg: Cfg):
    D = cfg.D
    GHD = cfg.G * cfg.HD  # head-dims per core
    cosT, sinT = _rope_tables(cfg)
    bias4 = (np.asarray(b_proj, np.float32) / GS).astype(np.float32)

    xT = [
        np.ascontiguousarray(np.asarray(x[b], np.float32).T).astype(ml_dtypes.bfloat16)
        for b in range(cfg.B)
    ]
    wqkvT, wprojT = [], []
    for g in range(GS):
        sl = slice(g * GHD, (g + 1) * GHD)
        wq = w_qkv[0:D][sl]
        wk = w_qkv[D : 2 * D][sl]
        wv = w_qkv[2 * D : 3 * D][sl]
        wqkvT.append(
            np.ascontiguousarray(
                np.concatenate([wq, wk, wv], axis=0).T.astype(np.float32)
            ).astype(ml_dtypes.bfloat16)
        )
        wprojT.append(
            np.ascontiguousarray(w_proj[:, sl].T.astype(np.float32)).astype(
                ml_dtypes.bfloat16
            )
        )

    in_maps = []
    for c in range(NCORES):
        b, g = divmod(c, GS)
        in_maps.append(
            {
                "xT": xT[b],
                "wqkvT": wqkvT[g],
                "wprojT": wprojT[g],
                "bias4": bias4,
                "cosT": cosT,
                "sinT": sinT,
            }
        )
    return in_maps


def assemble(results, cfg: Cfg):
    ys = []
    for b in range(cfg.B):
        ybT = np.concatenate(
            [results[b * GS + r]["out"] for r in range(GS)], axis=0
        )  # [D, N]
        ys.append(ybT.T)
    return np.stack(ys).astype(np.float32)


_NC_CACHE = {}


def _get_nc(cfg: Cfg):
    if cfg not in _NC_CACHE:
        _NC_CACHE[cfg] = build(cfg)
    return _NC_CACHE[cfg]


LAST_RESULT = None


def kernel(x, w_qkv, w_proj, b_proj):
    global LAST_RESULT
    cfg = FULL
    nc = _get_nc(cfg)
    in_maps = prepare_in_maps(
        np.asarray(x), np.asarray(w_qkv), np.asarray(w_proj), np.asarray(b_proj), cfg
    )
    res = run_bass_kernel_spmd(nc, in_maps, core_ids=list(range(NCORES)))
    LAST_RESULT = res
    return assemble(res.results, cfg)


# revision 5
# speedup vs baseline: 1.5204x; 1.5204x over previous
"""Multi-head attention (RoPE + SDPA + output projection) on 8 Trainium2 cores.

Problem: nn_Attention_80152679678101
  x[2,2048,2048] @ w_qkv.T -> rope(q,k) -> softmax(q k^T/sqrt(128)) v -> @ w_proj.T + b

Sharding: core c -> (batch b = c//4, head-group g = c%4, 4 heads each);
tensor-parallel heads within each 4-core batch group.

Dataflow is fully transposed so every matmul has its contraction dim on SBUF
partitions with no on-chip transposes: the host feeds x^T, w_qkv_slice^T and a
head-permuted w_proj^T (bf16). Stages per core:
  A) qkv^T: Q^T,K^T as [head_dim, n] (lhsT=w^T, rhs=x^T); V as [n, head_dim]
     (lhsT=x^T, rhs=w_v^T)
  B) RoPE on Q^T/K^T fused into the projection epilogue: half-swap via
     SBUF->SBUF DMA + 3 DVE ops against host-precomputed cos/sin tables
     (sign folded into the sin table)
  C) per head: S^T = K^T-tiles.T @ Q^T (PE) -> exp via ACT on [128,1024]
     chunks (1/sqrt(128) scale folded; no max-subtraction, scores are ~N(0,1)
     so fp32 exp is safe) -> softmax denominators via an all-ones stationary
     matmul (yields l[q] replicated across all 128 partitions) ->
     O'^T = V.T @ P^T -> reciprocal_approx_fast + scale
  D) per-head AllGather of the normalized head outputs (overlaps the next
     head's attention); each core then computes the full-contraction output
     projection for its own q-slice, selected with a partition_id-dependent
     dynamic DMA offset, + bias. No reduce needed afterwards.
"""

import os

# Never attempt NTFF tracing unless a dev harness explicitly opts in: the
# trace path uploads artifacts to S3, which is unavailable when grading.
if "KERNEL_ALLOW_TRACE" not in os.environ:
    os.environ["BASS_NEVER_TRACE"] = "1"

from contextlib import ExitStack
from dataclasses import dataclass

import ml_dtypes
import numpy as np

import concourse.bass as bass
import concourse.mybir as mybir
import concourse.tile as tile
from concourse import bacc
from concourse.bass import ds
from concourse.bass_utils import run_bass_kernel_spmd

BF16 = mybir.dt.bfloat16
FP32 = mybir.dt.float32
AF = mybir.ActivationFunctionType

NCORES = 8
GS = 4  # tensor-parallel group size (cores per batch)
REPLICA_GROUPS = [[0, 1, 2, 3], [4, 5, 6, 7]]
P = 128  # SBUF partitions
ROPE_BASE = 10000.0


@dataclass(frozen=True)
class Cfg:
    B: int = 2
    N: int = 2048  # sequence length
    D: int = 2048  # model dim
    H: int = 16  # total heads

    @property
    def HD(self):  # head dim
        return self.D // self.H

    @property
    def G(self):  # heads per core
        return self.H // GS

    @property
    def E(self):  # local qkv output rows
        return 3 * self.G * self.HD

    @property
    def KT(self):  # contraction tiles over D
        return self.D // P

    @property
    def SEQT(self):  # sequence tiles of 128
        return self.N // P

    @property
    def NT(self):  # matmul moving free-dim tile (one PSUM bank of fp32)
        return min(512, self.N)

    @property
    def QT(self):  # moving-dim tiles over N
        return self.N // self.NT

    @property
    def QH(self):  # exp chunk width (2 PSUM banks)
        return min(1024, self.N)

    @property
    def OT(self):  # output-projection row tiles
        return self.D // P

    @property
    def QS(self):  # per-core q-slice width for the output projection
        return self.N // GS


FULL = Cfg()


def build(cfg: Cfg) -> bass.Bass:
    assert cfg.HD == P, "rope/half-swap layout assumes head_dim == 128"
    G, E, KT, SEQT, NT, QT, QH, OT, QS = (
        cfg.G, cfg.E, cfg.KT, cfg.SEQT, cfg.NT, cfg.QT, cfg.QH, cfg.OT, cfg.QS,
    )
    N, D = cfg.N, cfg.D
    KT16 = 4 * G  # proj contraction tiles (= gathered head-dim tiles)
    HALVES = N // QH
    SUBS = QH // NT
    VOFF = 2 * G * P  # column offset of the v block in wqkvT
    scale = 1.0 / float(np.sqrt(cfg.HD))

    nc = bacc.Bacc(
        "TRN2", target_bir_lowering=False, debug=False, num_devices=NCORES
    )

    xT = nc.dram_tensor("xT", [D, N], BF16, kind="ExternalInput")
    wqkvT = nc.dram_tensor("wqkvT", [D, E], BF16, kind="ExternalInput")
    wprojT = nc.dram_tensor("wprojT", [D, D], BF16, kind="ExternalInput")
    biasd = nc.dram_tensor("biasd", [D], FP32, kind="ExternalInput")
    cosT = nc.dram_tensor("cosT", [P, N], BF16, kind="ExternalInput")
    sinT = nc.dram_tensor("sinT", [P, N], BF16, kind="ExternalInput")
    out = nc.dram_tensor("out", [D, QS], FP32, kind="ExternalOutput")

    with tile.TileContext(nc) as tc, ExitStack() as ctx:
        dram = ctx.enter_context(tc.tile_pool(name="dram", bufs=1, space="DRAM"))
        const = ctx.enter_context(tc.tile_pool(name="const", bufs=1))

        cos_sb = const.tile([P, N], BF16)
        sin_sb = const.tile([P, N], BF16)
        ones_sb = const.tile([P, P], BF16)
        bias_sb = const.tile([P, OT], FP32)
        nc.sync.dma_start(cos_sb[:], cosT[:])
        nc.sync.dma_start(sin_sb[:], sinT[:])
        nc.vector.memset(ones_sb[:], 1.0)
        nc.sync.dma_start(bias_sb[:], biasd.ap().rearrange("(t p) -> p t", p=P))

        # q-slice offset for the output projection: rank within the
        # 4-core replica group
        qoff = (nc.sync.partition_id() % GS) * QS

        # live through stages A-C
        qk_pool = ctx.enter_context(tc.tile_pool(name="qk", bufs=1))
        v_pool = ctx.enter_context(tc.tile_pool(name="v", bufs=1))
        qt_sb = [qk_pool.tile([P, N], BF16, name=f"q_h{j}") for j in range(G)]
        kt_sb = [qk_pool.tile([P, N], BF16, name=f"k_h{j}") for j in range(G)]
        v_sb = v_pool.tile([P, SEQT, G * P], BF16)

        # ---- stage A: qkv projection (+ rope fused into the epilogue) ----
        with (
            tc.tile_pool(name="inw", bufs=1) as in_pool,
            tc.tile_pool(name="rope", bufs=3) as rope_pool,
            tc.tile_pool(name="ps_a", bufs=6, space="PSUM") as ps_a,
        ):
            xT_sb = [in_pool.tile([P, N], BF16, name=f"xT{k}") for k in range(KT)]
            wq_sb = [in_pool.tile([P, E], BF16, name=f"wq{k}") for k in range(KT)]
            for k in range(KT):
                nc.sync.dma_start(xT_sb[k][:], xT[k * P : (k + 1) * P, :])
                nc.sync.dma_start(wq_sb[k][:], wqkvT[k * P : (k + 1) * P, :])

            # A2: V natural layout [n, G*HD]
            for s in range(SEQT):
                ps = ps_a.tile([P, G * P], FP32, name="ps_v", tag="ps")
                for k in range(KT):
                    nc.tensor.matmul(
                        ps[:],
                        xT_sb[k][:, s * P : (s + 1) * P],
                        wq_sb[k][:, VOFF : VOFF + G * P],
                        start=(k == 0),
                        stop=(k == KT - 1),
                    )
                nc.vector.tensor_copy(v_sb[:, s, :], ps[:])

            # A1: Q^T / K^T per head-dim tile, rope epilogue per NT chunk
            for e in range(2 * G):
                dst = qt_sb[e] if e < G else kt_sb[e - G]
                for q in range(QT):
                    ps = ps_a.tile([P, NT], FP32, name="ps_qk", tag="ps")
                    for k in range(KT):
                        nc.tensor.matmul(
                            ps[:],
                            wq_sb[k][:, e * P : (e + 1) * P],
                            xT_sb[k][:, q * NT : (q + 1) * NT],
                            start=(k == 0),
                            stop=(k == KT - 1),
                        )
                    sl = slice(q * NT, (q + 1) * NT)
                    raw = rope_pool.tile([P, NT], FP32, name="raw")
                    nc.vector.tensor_copy(raw[:], ps[:])
                    # rotate-half: swp = [raw[64:], raw[:64]]
                    swp = rope_pool.tile([P, NT], FP32, name="swp")
                    h = P // 2
                    nc.sync.dma_start(swp[0:h, :], raw[h:P, :])
                    nc.sync.dma_start(swp[h:P, :], raw[0:h, :])
                    tmp = rope_pool.tile([P, NT], FP32, name="tmp")
                    nc.vector.tensor_mul(tmp[:], swp[:], sin_sb[:, sl])
                    nc.vector.tensor_mul(raw[:], raw[:], cos_sb[:, sl])
                    nc.vector.tensor_add(dst[:, sl], raw[:], tmp[:])

        # proj weights: loaded into the space freed by stage A; the DMA is
        # dependency-gated on the last stage-A readers and overlaps attention
        wp_pool = ctx.enter_context(tc.tile_pool(name="wp", bufs=1))
        af_pool = ctx.enter_context(tc.tile_pool(name="af", bufs=1))
        wp_sb = wp_pool.tile([P, KT16, D], BF16)
        af_sb = af_pool.tile([P, KT16, QS], BF16)
        for t in range(KT16):
            nc.sync.dma_start(
                wp_sb[:, t, :],
                wprojT.ap().rearrange("(t p) d -> t p d", p=P)[t],
            )

        at_dram = [dram.tile([P, N], BF16, name=f"at_d{j}") for j in range(G)]
        af_dram = [dram.tile([GS * P, N], BF16, name=f"af_d{j}") for j in range(G)]

        # ---- stage C: attention per head, AllGather per head ----
        with (
            tc.tile_pool(name="pt", bufs=1) as pt_pool,
            tc.tile_pool(name="atst", bufs=4) as at_pool,
            tc.tile_pool(name="rb", bufs=2) as rb_pool,
            tc.tile_pool(name="ps_s", bufs=2, space="PSUM") as ps_s,
            tc.tile_pool(name="ps_l", bufs=2, space="PSUM") as ps_l,
            tc.tile_pool(name="ps_o", bufs=2, space="PSUM") as ps_o,
        ):
            for j in range(G):
                for hh in range(HALVES):
                    h0 = hh * QH
                    pt = pt_pool.tile([P, SEQT, QH], BF16, name="pt", tag="pt")
                    # scores S^T[k, q] + exp, [128, QH] chunks
                    for s in range(SEQT):
                        ps = ps_s.tile([P, QH], FP32, name="ps_sc", tag="sc")
                        for u in range(SUBS):
                            nc.tensor.matmul(
                                ps[:, u * NT : (u + 1) * NT],
                                kt_sb[j][:, s * P : (s + 1) * P],
                                qt_sb[j][:, h0 + u * NT : h0 + (u + 1) * NT],
                                start=True,
                                stop=True,
                            )
                        nc.scalar.activation(pt[:, s, :], ps[:], AF.Exp, scale=scale)
                    # denominators (ones-matmul -> replicated over partitions)
                    # and O'^T accumulation, then normalize
                    for u in range(SUBS):
                        q0 = h0 + u * NT
                        psl = ps_l.tile([P, NT], FP32, name="ps_lb", tag="lb")
                        pso = ps_o.tile([P, NT], FP32, name="ps_ov", tag="ov")
                        usl = slice(u * NT, (u + 1) * NT)
                        for s in range(SEQT):
                            nc.tensor.matmul(
                                psl[:], ones_sb[:], pt[:, s, usl],
                                start=(s == 0), stop=(s == SEQT - 1),
                            )
                            nc.tensor.matmul(
                                pso[:], v_sb[:, s, j * P : (j + 1) * P],
                                pt[:, s, usl],
                                start=(s == 0), stop=(s == SEQT - 1),
                            )
                        rb = rb_pool.tile([P, NT], FP32, name="rb")
                        nc.vector.reciprocal_approx_fast(rb[:], psl[:])
                        at = at_pool.tile([P, NT], BF16, name="at", tag="at")
                        nc.vector.tensor_mul(at[:], pso[:], rb[:])
                        nc.sync.dma_start(
                            at_dram[j][:, q0 : q0 + NT], at[:]
                        )
                # gather this head's outputs across the group; rows land in
                # rank order = head-dim blocks of heads {g'*G + j}
                nc.gpsimd.collective_compute(
                    "AllGather",
                    mybir.AluOpType.bypass,
                    replica_groups=REPLICA_GROUPS,
                    ins=[at_dram[j][:]],
                    outs=[af_dram[j][:]],
                )
                for gp in range(GS):
                    nc.sync.dma_start(
                        af_sb[:, j * GS + gp, :],
                        af_dram[j][gp * P : (gp + 1) * P, ds(qoff, QS)],
                    )

        # ---- stage D: output projection (full contraction, own q-slice) ----
        with (
            tc.tile_pool(name="ystg", bufs=4) as y_pool,
            tc.tile_pool(name="ps_y", bufs=8, space="PSUM") as ps_y,
        ):
            for o in range(OT):
                ps = ps_y.tile([P, QS], FP32, name="ps_yt", tag="y")
                for t in range(KT16):
                    nc.tensor.matmul(
                        ps[:],
                        wp_sb[:, t, o * P : (o + 1) * P],
                        af_sb[:, t, :],
                        start=(t == 0),
                        stop=(t == KT16 - 1),
                    )
                ystg = y_pool.tile([P, QS], FP32, name="ystg")
                nc.scalar.activation(
                    ystg[:], ps[:], AF.Identity, bias=bias_sb[:, o : o + 1]
                )
                nc.sync.dma_start(out[o * P : (o + 1) * P, :], ystg[:])

    nc.compile()
    return nc


def _rope_tables(cfg: Cfg):
    hd = cfg.HD
    inv_freq = 1.0 / (
        ROPE_BASE ** (np.arange(0, hd, 2, dtype=np.float32) / np.float32(hd))
    )
    ang = np.arange(cfg.N, dtype=np.float32)[:, None] * inv_freq[None, :]  # [N, hd/2]
    c = np.cos(ang).T  # [hd/2, N]
    s = np.sin(ang).T
    cosT = np.concatenate([c, c], axis=0)
    sinT = np.concatenate([-s, s], axis=0)
    return (
        np.ascontiguousarray(cosT).astype(ml_dtypes.bfloat16),
        np.ascontiguousarray(sinT).astype(ml_dtypes.bfloat16),
    )


def prepare_in_maps(x, w_qkv, w_proj, b_proj, cfg: Cfg):
    D = cfg.D
    GHD = cfg.G * cfg.HD  # head-dims per core
    cosT, sinT = _rope_tables(cfg)
    bias = np.ascontiguousarray(np.asarray(b_proj, np.float32))

    xT = [
        np.ascontiguousarray(np.asarray(x[b], np.float32).T).astype(ml_dtypes.bfloat16)
        for b in range(cfg.B)
    ]
    wqkvT = []
    for g in range(GS):
        sl = slice(g * GHD, (g + 1) * GHD)
        wq = w_qkv[0:D][sl]
        wk = w_qkv[D : 2 * D][sl]
        wv = w_qkv[2 * D : 3 * D][sl]
        wqkvT.append(
            np.ascontiguousarray(
                np.concatenate([wq, wk, wv], axis=0).T.astype(np.float32)
            ).astype(ml_dtypes.bfloat16)
        )
    # w_proj^T with rows permuted to the AllGather head order:
    # kt16 = j*GS + g'  ->  head g'*G + j
    perm = [gp * cfg.G + j for j in range(cfg.G) for gp in range(GS)]
    wpT = np.asarray(w_proj, np.float32).T.reshape(cfg.H, cfg.HD, D)[perm]
    wprojT = np.ascontiguousarray(wpT.reshape(D, D)).astype(ml_dtypes.bfloat16)

    in_maps = []
    for c in range(NCORES):
        b, g = divmod(c, GS)
        in_maps.append(
            {
                "xT": xT[b],
                "wqkvT": wqkvT[g],
                "wprojT": wprojT,
                "biasd": bias,
                "cosT": cosT,
                "sinT": sinT,
            }
        )
    return in_maps


def assemble(results, cfg: Cfg):
    ys = []
    for b in range(cfg.B):
        ybT = np.concatenate(
            [results[b * GS + r]["out"] for r in range(GS)], axis=1
        )  # [D, N]
        ys.append(ybT.T)
    return np.stack(ys).astype(np.float32)


_NC_CACHE = {}


def _get_nc(cfg: Cfg):
    if cfg not in _NC_CACHE:
        _NC_CACHE[cfg] = build(cfg)
    return _NC_CACHE[cfg]


LAST_RESULT = None


def kernel(x, w_qkv, w_proj, b_proj):
    global LAST_RESULT
    cfg = FULL
    nc = _get_nc(cfg)
    in_maps = prepare_in_maps(
        np.asarray(x), np.asarray(w_qkv), np.asarray(w_proj), np.asarray(b_proj), cfg
    )
    res = run_bass_kernel_spmd(nc, in_maps, core_ids=list(range(NCORES)))
    LAST_RESULT = res
    return assemble(res.results, cfg)


# revision 7
# speedup vs baseline: 1.5513x; 1.0203x over previous
"""Multi-head attention (RoPE + SDPA + output projection) on 8 Trainium2 cores.

Problem: nn_Attention_80152679678101
  x[2,2048,2048] @ w_qkv.T -> rope(q,k) -> softmax(q k^T/sqrt(128)) v -> @ w_proj.T + b

Sharding: core c -> (batch b = c//4, head-group g = c%4, 4 heads each);
tensor-parallel heads within each 4-core batch group.

Dataflow is fully transposed so every matmul has its contraction dim on SBUF
partitions with no on-chip transposes: the host feeds x^T, w_qkv_slice^T and a
head-permuted w_proj^T (bf16). Stages per core:
  A) qkv^T: Q^T,K^T as [head_dim, n] (lhsT=w^T, rhs=x^T); V as [n, head_dim]
     (lhsT=x^T, rhs=w_v^T)
  B) RoPE on Q^T/K^T fused into the projection epilogue: half-swap via
     SBUF->SBUF DMA + 3 DVE ops against host-precomputed cos/sin tables
     (sign folded into the sin table)
  C) per head: S^T = K^T-tiles.T @ Q^T (PE) -> exp via ACT on [128,1024]
     chunks (1/sqrt(128) scale folded; no max-subtraction, scores are ~N(0,1)
     so fp32 exp is safe) -> softmax denominators via an all-ones stationary
     matmul (yields l[q] replicated across all 128 partitions) ->
     O'^T = V.T @ P^T -> reciprocal_approx_fast + scale
  D) per-head AllGather of the normalized head outputs (overlaps the next
     head's attention); each core then computes the full-contraction output
     projection for its own q-slice, selected with a partition_id-dependent
     dynamic DMA offset, + bias. No reduce needed afterwards.
"""

import os

# Never attempt NTFF tracing unless a dev harness explicitly opts in: the
# trace path uploads artifacts to S3, which is unavailable when grading.
if "KERNEL_ALLOW_TRACE" not in os.environ:
    os.environ["BASS_NEVER_TRACE"] = "1"

from contextlib import ExitStack
from dataclasses import dataclass

import ml_dtypes
import numpy as np

import concourse.bass as bass
import concourse.mybir as mybir
import concourse.tile as tile
from concourse import bacc
from concourse.bass import ds
from concourse.bass_utils import run_bass_kernel_spmd

BF16 = mybir.dt.bfloat16
FP32 = mybir.dt.float32
AF = mybir.ActivationFunctionType

NCORES = 8
GS = 4  # tensor-parallel group size (cores per batch)
REPLICA_GROUPS = [[0, 1, 2, 3], [4, 5, 6, 7]]
P = 128  # SBUF partitions
ROPE_BASE = 10000.0


@dataclass(frozen=True)
class Cfg:
    B: int = 2
    N: int = 2048  # sequence length
    D: int = 2048  # model dim
    H: int = 16  # total heads

    @property
    def HD(self):  # head dim
        return self.D // self.H

    @property
    def G(self):  # heads per core
        return self.H // GS

    @property
    def E(self):  # local qkv output rows
        return 3 * self.G * self.HD

    @property
    def KT(self):  # contraction tiles over D
        return self.D // P

    @property
    def SEQT(self):  # sequence tiles of 128
        return self.N // P

    @property
    def NT(self):  # matmul moving free-dim tile (one PSUM bank of fp32)
        return min(512, self.N)

    @property
    def QT(self):  # moving-dim tiles over N
        return self.N // self.NT

    @property
    def QH(self):  # exp chunk width (2 PSUM banks)
        return min(1024, self.N)

    @property
    def OT(self):  # output-projection row tiles
        return self.D // P

    @property
    def QS(self):  # per-core q-slice width for the output projection
        return self.N // GS


FULL = Cfg()


def build(cfg: Cfg) -> bass.Bass:
    assert cfg.HD == P, "rope/half-swap layout assumes head_dim == 128"
    G, E, KT, SEQT, NT, QT, QH, OT, QS = (
        cfg.G, cfg.E, cfg.KT, cfg.SEQT, cfg.NT, cfg.QT, cfg.QH, cfg.OT, cfg.QS,
    )
    N, D = cfg.N, cfg.D
    KT16 = 4 * G  # proj contraction tiles (= gathered head-dim tiles)
    HALVES = N // QH
    SUBS = QH // NT
    VOFF = 2 * G * P  # column offset of the v block in wqkvT
    scale = 1.0 / float(np.sqrt(cfg.HD))

    nc = bacc.Bacc(
        "TRN2", target_bir_lowering=False, debug=False, num_devices=NCORES
    )

    xT = nc.dram_tensor("xT", [D, N], BF16, kind="ExternalInput")
    wqkvT = nc.dram_tensor("wqkvT", [D, E], BF16, kind="ExternalInput")
    wprojT = nc.dram_tensor("wprojT", [D, D], BF16, kind="ExternalInput")
    biasd = nc.dram_tensor("biasd", [D], FP32, kind="ExternalInput")
    cosT = nc.dram_tensor("cosT", [P, N], BF16, kind="ExternalInput")
    sinT = nc.dram_tensor("sinT", [P, N], BF16, kind="ExternalInput")
    out = nc.dram_tensor("out", [D, QS], FP32, kind="ExternalOutput")

    with tile.TileContext(nc) as tc, ExitStack() as ctx:
        dram = ctx.enter_context(tc.tile_pool(name="dram", bufs=1, space="DRAM"))
        const = ctx.enter_context(tc.tile_pool(name="const", bufs=1))

        cos_sb = const.tile([P, N], BF16)
        sin_sb = const.tile([P, N], BF16)
        ones_sb = const.tile([P, P], BF16)
        bias_sb = const.tile([P, OT], FP32)
        nc.sync.dma_start(cos_sb[:], cosT[:])
        nc.sync.dma_start(sin_sb[:], sinT[:])
        nc.vector.memset(ones_sb[:], 1.0)
        nc.sync.dma_start(bias_sb[:], biasd.ap().rearrange("(t p) -> p t", p=P))

        # q-slice offset for the output projection: rank within the
        # 4-core replica group
        qoff = (nc.sync.partition_id() % GS) * QS

        # live through stages A-C
        qk_pool = ctx.enter_context(tc.tile_pool(name="qk", bufs=1))
        v_pool = ctx.enter_context(tc.tile_pool(name="v", bufs=1))
        qt_sb = [qk_pool.tile([P, N], BF16, name=f"q_h{j}") for j in range(G)]
        kt_sb = [qk_pool.tile([P, N], BF16, name=f"k_h{j}") for j in range(G)]
        v_sb = v_pool.tile([P, SEQT, G * P], BF16)

        # ---- stage A: qkv projection (+ rope fused into the epilogue) ----
        with (
            tc.tile_pool(name="inw", bufs=1) as in_pool,
            tc.tile_pool(name="rope", bufs=3) as rope_pool,
            tc.tile_pool(name="ps_a", bufs=6, space="PSUM") as ps_a,
        ):
            xT_sb = in_pool.tile([P, KT, N], BF16)
            wq_sb = in_pool.tile([P, KT, E], BF16)
            xT_r = xT.ap().rearrange("(k p) n -> p k n", p=P)
            wq_r = wqkvT.ap().rearrange("(k p) e -> p k e", p=P)
            # qk-block weights first, then x by q-chunk, v-block last: the
            # first A1 matmul group only waits on ~4MB of input
            nc.sync.dma_start(wq_sb[:, :, 0:VOFF], wq_r[:, :, 0:VOFF])
            for q in range(QT):
                sl = slice(q * NT, (q + 1) * NT)
                nc.sync.dma_start(xT_sb[:, :, sl], xT_r[:, :, sl])
            nc.sync.dma_start(wq_sb[:, :, VOFF:E], wq_r[:, :, VOFF:E])

            # A1: Q^T / K^T per head-dim tile, rope epilogue per NT chunk
            for e in range(2 * G):
                dst = qt_sb[e] if e < G else kt_sb[e - G]
                for q in range(QT):
                    ps = ps_a.tile([P, NT], FP32, name="ps_qk", tag="ps")
                    for k in range(KT):
                        nc.tensor.matmul(
                            ps[:],
                            wq_sb[:, k, e * P : (e + 1) * P],
                            xT_sb[:, k, q * NT : (q + 1) * NT],
                            start=(k == 0),
                            stop=(k == KT - 1),
                        )
                    sl = slice(q * NT, (q + 1) * NT)
                    raw = rope_pool.tile([P, NT], FP32, name="raw")
                    nc.vector.tensor_copy(raw[:], ps[:])
                    # rotate-half: swp = [raw[64:], raw[:64]]
                    swp = rope_pool.tile([P, NT], FP32, name="swp")
                    h = P // 2
                    nc.sync.dma_start(swp[0:h, :], raw[h:P, :])
                    nc.sync.dma_start(swp[h:P, :], raw[0:h, :])
                    tmp = rope_pool.tile([P, NT], FP32, name="tmp")
                    nc.vector.tensor_mul(tmp[:], swp[:], sin_sb[:, sl])
                    nc.vector.tensor_mul(raw[:], raw[:], cos_sb[:, sl])
                    nc.vector.tensor_add(dst[:, sl], raw[:], tmp[:])

            # A2: V natural layout [n, G*HD]
            for s in range(SEQT):
                ps = ps_a.tile([P, G * P], FP32, name="ps_v", tag="ps")
                for k in range(KT):
                    nc.tensor.matmul(
                        ps[:],
                        xT_sb[:, k, s * P : (s + 1) * P],
                        wq_sb[:, k, VOFF : VOFF + G * P],
                        start=(k == 0),
                        stop=(k == KT - 1),
                    )
                nc.vector.tensor_copy(v_sb[:, s, :], ps[:])

        # proj weights: loaded into the space freed by stage A; the DMA is
        # dependency-gated on the last stage-A readers and overlaps attention
        wp_pool = ctx.enter_context(tc.tile_pool(name="wp", bufs=1))
        af_pool = ctx.enter_context(tc.tile_pool(name="af", bufs=1))
        wp_sb = wp_pool.tile([P, KT16, D], BF16)
        af_sb = af_pool.tile([P, KT16, QS], BF16)
        for t in range(KT16):
            nc.sync.dma_start(
                wp_sb[:, t, :],
                wprojT.ap().rearrange("(t p) d -> t p d", p=P)[t],
            )

        at_dram = [dram.tile([P, N], BF16, name=f"at_d{j}") for j in range(G)]
        af_dram = [dram.tile([GS * P, N], BF16, name=f"af_d{j}") for j in range(G)]

        # ---- stage C: attention per head, AllGather per head ----
        with (
            tc.tile_pool(name="pt", bufs=1) as pt_pool,
            tc.tile_pool(name="atst", bufs=4) as at_pool,
            tc.tile_pool(name="rb", bufs=2) as rb_pool,
            tc.tile_pool(name="ps_s", bufs=2, space="PSUM") as ps_s,
            tc.tile_pool(name="ps_l", bufs=2, space="PSUM") as ps_l,
            tc.tile_pool(name="ps_o", bufs=2, space="PSUM") as ps_o,
        ):
            for j in range(G):
                for hh in range(HALVES):
                    h0 = hh * QH
                    pt = pt_pool.tile([P, SEQT, QH], BF16, name="pt", tag="pt")
                    # scores S^T[k, q] + exp, [128, QH] chunks
                    for s in range(SEQT):
                        ps = ps_s.tile([P, QH], FP32, name="ps_sc", tag="sc")
                        for u in range(SUBS):
                            nc.tensor.matmul(
                                ps[:, u * NT : (u + 1) * NT],
                                kt_sb[j][:, s * P : (s + 1) * P],
                                qt_sb[j][:, h0 + u * NT : h0 + (u + 1) * NT],
                                start=True,
                                stop=True,
                            )
                        nc.scalar.activation(pt[:, s, :], ps[:], AF.Exp, scale=scale)
                    # denominators (ones-matmul -> replicated over partitions)
                    # and O'^T accumulation, then normalize
                    for u in range(SUBS):
                        q0 = h0 + u * NT
                        psl = ps_l.tile([P, NT], FP32, name="ps_lb", tag="lb")
                        pso = ps_o.tile([P, NT], FP32, name="ps_ov", tag="ov")
                        usl = slice(u * NT, (u + 1) * NT)
                        for s in range(SEQT):
                            nc.tensor.matmul(
                                psl[:], ones_sb[:], pt[:, s, usl],
                                start=(s == 0), stop=(s == SEQT - 1),
                            )
                            nc.tensor.matmul(
                                pso[:], v_sb[:, s, j * P : (j + 1) * P],
                                pt[:, s, usl],
                                start=(s == 0), stop=(s == SEQT - 1),
                            )
                        rb = rb_pool.tile([P, NT], FP32, name="rb")
                        nc.vector.reciprocal_approx_fast(rb[:], psl[:])
                        at = at_pool.tile([P, NT], BF16, name="at", tag="at")
                        nc.vector.tensor_mul(at[:], pso[:], rb[:])
                        nc.sync.dma_start(
                            at_dram[j][:, q0 : q0 + NT], at[:]
                        )
                # gather this head's outputs across the group; rows land in
                # rank order = head-dim blocks of heads {g'*G + j}
                nc.gpsimd.collective_compute(
                    "AllGather",
                    mybir.AluOpType.bypass,
                    replica_groups=REPLICA_GROUPS,
                    ins=[at_dram[j][:]],
                    outs=[af_dram[j][:]],
                )
                for gp in range(GS):
                    nc.sync.dma_start(
                        af_sb[:, j * GS + gp, :],
                        af_dram[j][gp * P : (gp + 1) * P, ds(qoff, QS)],
                    )

        # ---- stage D: output projection (full contraction, own q-slice) ----
        with (
            tc.tile_pool(name="ystg", bufs=4) as y_pool,
            tc.tile_pool(name="ps_y", bufs=1, space="PSUM") as ps_y,
        ):
            OCH = 8 if OT % 8 == 0 else OT
            for oc in range(0, OT, OCH):
                pss = [
                    ps_y.tile([P, QS], FP32, name=f"ps_y{o}", tag=f"y{o - oc}")
                    for o in range(oc, oc + OCH)
                ]
                # contraction-major so all head-j<G-1 matmuls issue before
                # the last head's AllGather has landed
                for t in range(KT16):
                    for i, o in enumerate(range(oc, oc + OCH)):
                        nc.tensor.matmul(
                            pss[i][:],
                            wp_sb[:, t, o * P : (o + 1) * P],
                            af_sb[:, t, :],
                            start=(t == 0),
                            stop=(t == KT16 - 1),
                        )
                for i, o in enumerate(range(oc, oc + OCH)):
                    ystg = y_pool.tile([P, QS], FP32, name="ystg")
                    nc.scalar.activation(
                        ystg[:], pss[i][:], AF.Identity, bias=bias_sb[:, o : o + 1]
                    )
                    nc.sync.dma_start(out[o * P : (o + 1) * P, :], ystg[:])

    nc.compile()
    return nc


def _rope_tables(cfg: Cfg):
    hd = cfg.HD
    inv_freq = 1.0 / (
        ROPE_BASE ** (np.arange(0, hd, 2, dtype=np.float32) / np.float32(hd))
    )
    ang = np.arange(cfg.N, dtype=np.float32)[:, None] * inv_freq[None, :]  # [N, hd/2]
    c = np.cos(ang).T  # [hd/2, N]
    s = np.sin(ang).T
    cosT = np.concatenate([c, c], axis=0)
    sinT = np.concatenate([-s, s], axis=0)
    return (
        np.ascontiguousarray(cosT).astype(ml_dtypes.bfloat16),
        np.ascontiguousarray(sinT).astype(ml_dtypes.bfloat16),
    )


def prepare_in_maps(x, w_qkv, w_proj, b_proj, cfg: Cfg):
    D = cfg.D
    GHD = cfg.G * cfg.HD  # head-dims per core
    cosT, sinT = _rope_tables(cfg)
    bias = np.ascontiguousarray(np.asarray(b_proj, np.float32))

    xT = [
        np.ascontiguousarray(np.asarray(x[b], np.float32).T).astype(ml_dtypes.bfloat16)
        for b in range(cfg.B)
    ]
    wqkvT = []
    for g in range(GS):
        sl = slice(g * GHD, (g + 1) * GHD)
        wq = w_qkv[0:D][sl]
        wk = w_qkv[D : 2 * D][sl]
        wv = w_qkv[2 * D : 3 * D][sl]
        wqkvT.append(
            np.ascontiguousarray(
                np.concatenate([wq, wk, wv], axis=0).T.astype(np.float32)
            ).astype(ml_dtypes.bfloat16)
        )
    # w_proj^T with rows permuted to the AllGather head order:
    # kt16 = j*GS + g'  ->  head g'*G + j
    perm = [gp * cfg.G + j for j in range(cfg.G) for gp in range(GS)]
    wpT = np.asarray(w_proj, np.float32).T.reshape(cfg.H, cfg.HD, D)[perm]
    wprojT = np.ascontiguousarray(wpT.reshape(D, D)).astype(ml_dtypes.bfloat16)

    in_maps = []
    for c in range(NCORES):
        b, g = divmod(c, GS)
        in_maps.append(
            {
                "xT": xT[b],
                "wqkvT": wqkvT[g],
                "wprojT": wprojT,
                "biasd": bias,
                "cosT": cosT,
                "sinT": sinT,
            }
        )
    return in_maps


def assemble(results, cfg: Cfg):
    ys = []
    for b in range(cfg.B):
        ybT = np.concatenate(
            [results[b * GS + r]["out"] for r in range(GS)], axis=1
        )  # [D, N]
        ys.append(ybT.T)
    return np.stack(ys).astype(np.float32)


_NC_CACHE = {}


def _get_nc(cfg: Cfg):
    if cfg not in _NC_CACHE:
        _NC_CACHE[cfg] = build(cfg)
    return _NC_CACHE[cfg]


LAST_RESULT = None


def kernel(x, w_qkv, w_proj, b_proj):
    global LAST_RESULT
    cfg = FULL
    nc = _get_nc(cfg)
    in_maps = prepare_in_maps(
        np.asarray(x), np.asarray(w_qkv), np.asarray(w_proj), np.asarray(b_proj), cfg
    )
    res = run_bass_kernel_spmd(nc, in_maps, core_ids=list(range(NCORES)))
    LAST_RESULT = res
    return assemble(res.results, cfg)
